# revision 1
# baseline (speedup 1.0000x reference)
"""Betti3D loss kernel for Trainium2 (8 NeuronCores, data-parallel over batch).

Reference computation (see problem):
    p_down  = trilinear_resize(p_hat, (32, 32, 8))   # [B, C, 32, 32, 8]
    conf[b] = max(p_down[b, struct_id])
    out     = sum((1 - conf) * betti_error) / B

With input [B, C, 160, 160, 64] -> (32, 32, 8) the resize scales are exactly
(5, 5, 8), so with torch/jax half-pixel centers the source coordinates are:
    D axis: 5*i + 2      (weight exactly 0 -> pure gather)
    H axis: 5*j + 2      (weight exactly 0 -> pure gather)
    W axis: 8*k + 3.5    (weight exactly 0.5 -> 0.5*(x[8k+3] + x[8k+4]))
Therefore
    p_down[b, c, i, j, k] = 0.5 * (x[b,c,5i+2,5j+2,8k+3] + x[b,c,5i+2,5j+2,8k+4])
and conf[b] = 0.5 * max_{i,j,k} (x[...,8k+3] + x[...,8k+4]).  Since scaling by
0.5 commutes with max (and is exact in fp32), the device kernel computes
max(a+b) and the host multiplies by 0.5, reproducing the reference bit-exactly.

Per-core kernel (one batch sample per core):
  - one strided DMA gathers the 32x32 needed rows of 64 floats (256 KB) of
    channel struct_id into SBUF [128, 512]
  - DVE: add of the two W-phases (x[...,3::8] + x[...,4::8]) -> [128, 8, 8]
  - DVE: max-reduce over free dim -> [128, 1]
  - DMA out 128 partition maxima; host finishes max/mean over 8*128 values.

betti_error is 1 only for struct_id == 2 ('Myo'); for the other structures the
loss is exactly 0 and no device work is needed.
"""

import os

import numpy as np

_TARGETS = ((1, 0, 0), (1, 0, 0), (1, 1, 0), (1, 0, 0))
_BETTI_FALLBACK = (1, 0, 0)

_N_CORES = 8
_IN_SHAPE = (4, 160, 160, 64)  # per-sample [C, D, H, W]

_module_cache: dict = {}
LAST_RESULTS = None  # BassKernelResults of the most recent device run


def _patch_tail_drain():
    """walrus in this image rejects >1 sem wait on the SP tail Drain
    ("Too many sync wait commands", CoreV3GenImpl setupSyncWait).  Split
    Tile's single multi-wait tail drain into one drain per semaphore."""
    import concourse.tile as tile
    from concourse import mybir
    from concourse.vector_clock import ScopedClock

    if getattr(tile.TileContext, "_betti_drain_patch", False):
        return

    def _drain_and_barrier(self, tick_clock, wait_clock):
        drain_inst = self.nc.sync.drain()
        wait_clock.add_sem_waits(
            drain_inst.ins, ScopedClock({None: tick_clock.global_clock})
        )
        si = drain_inst.ins.sync_info
        if si is not None and si.on_wait and len(si.on_wait) > 1:
            waits = list(si.on_wait)
            drain_inst.ins.sync_info = mybir.SyncInfo(
                on_wait=[waits[0]], on_update=list(si.on_update or [])
            )
            for w in waits[1:]:
                extra = self.nc.sync.drain()
                extra.ins.sync_info = mybir.SyncInfo(on_wait=[w], on_update=[])
        self.nc.all_engine_barrier()
        assert self.sems is not None
        popped = self.nc._tile_sem_poison_stack.pop()
        assert popped is self._sem_poison
        self.nc.clear_and_free_semaphores(list(self.sems.allocated().values()))
        self.nc.all_engine_barrier()

    tile.TileContext._drain_and_barrier = _drain_and_barrier
    tile.TileContext._betti_drain_patch = True


def _build(struct_id: int):
    import concourse.bass as bass
    import concourse.tile as tile
    from concourse import mybir

    _patch_tail_drain()

    nc = bass.Bass("TRN2", target_bir_lowering=False, debug=False,
                   num_devices=_N_CORES)
    x = nc.dram_tensor("x", list(_IN_SHAPE), mybir.dt.float32,
                       kind="ExternalInput").ap()
    o = nc.dram_tensor("o", [128], mybir.dt.float32,
                       kind="ExternalOutput").ap()
    with tile.TileContext(nc) as tc:
        with tc.tile_pool(name="p", bufs=1) as pool:
            # 1024 rows (i,j) x 64 floats; 8 rows per partition.
            t = pool.tile([128, 512], mybir.dt.float32)
            nc.sync.dma_start(t[:], x[struct_id, 2::5, 2::5, :])
            tv = t[:].rearrange("p (j w) -> p j w", w=64)
            scr = pool.tile([128, 64], mybir.dt.float32)
            sv = scr[:].rearrange("p (j k) -> p j k", k=8)
            red = pool.tile([128, 1], mybir.dt.float32)
            nc.vector.tensor_tensor(out=sv, in0=tv[:, :, 3::8],
                                    in1=tv[:, :, 4::8],
                                    op=mybir.AluOpType.add)
            nc.vector.reduce_max(red[:], scr[:], axis=mybir.AxisListType.X)
            nc.sync.dma_start(o[:], red[:])
    return nc


def kernel(p_hat: np.ndarray, struct_id) -> np.ndarray:
    global LAST_RESULTS
    sid = int(struct_id)
    target = _TARGETS[sid]
    betti_error = sum(abs(_BETTI_FALLBACK[k] - target[k]) for k in range(3))
    B = p_hat.shape[0]
    if betti_error == 0:
        return np.zeros((), dtype=p_hat.dtype)

    from concourse import bass_utils

    assert B == _N_CORES and tuple(p_hat.shape[1:]) == _IN_SHAPE, (
        f"kernel hardcoded for shape (8, 4, 160, 160, 64), got {p_hat.shape}"
    )
    if sid not in _module_cache:
        _module_cache[sid] = _build(sid)
    nc = _module_cache[sid]

    p_hat = np.ascontiguousarray(p_hat, dtype=np.float32)
    in_maps = [{"x": p_hat[b]} for b in range(B)]
    trace = bool(int(os.environ.get("BETTI_TRACE", "0")))
    res = bass_utils.run_bass_kernel_spmd(
        nc, in_maps, core_ids=list(range(_N_CORES)), trace=trace
    )
    LAST_RESULTS = res

    per_core = np.stack([r["o"] for r in res.results])        # [8, 128]
    m = per_core.max(axis=1).astype(np.float32)               # max of (a+b)
    conf = np.float32(0.5) * m                                # exact scaling
    total = np.sum((np.float32(1.0) - conf) * np.float32(betti_error),
                   dtype=np.float32)
    out = total / np.float32(max(B, 1))
    return np.asarray(out, dtype=p_hat.dtype)


# revision 3
# speedup vs baseline: 1.3911x; 1.3911x over previous
"""Betti3D loss kernel for Trainium2 (8 NeuronCores, data-parallel over batch).

Reference computation (see problem):
    p_down  = trilinear_resize(p_hat, (32, 32, 8))   # [B, C, 32, 32, 8]
    conf[b] = max(p_down[b, struct_id])
    out     = sum((1 - conf) * betti_error) / B

With input [B, C, 160, 160, 64] -> (32, 32, 8) the resize scales are exactly
(5, 5, 8), so with torch/jax half-pixel centers the source coordinates are:
    D axis: 5*i + 2      (weight exactly 0 -> pure gather)
    H axis: 5*j + 2      (weight exactly 0 -> pure gather)
    W axis: 8*k + 3.5    (weight exactly 0.5 -> 0.5*(x[8k+3] + x[8k+4]))
Therefore
    p_down[b, c, i, j, k] = 0.5 * (x[b,c,5i+2,5j+2,8k+3] + x[b,c,5i+2,5j+2,8k+4])
and conf[b] = 0.5 * max_{i,j,k} (x[...,8k+3] + x[...,8k+4]).  Since scaling by
0.5 commutes with max (and is exact in fp32), the device kernel computes
max(a+b) and the host multiplies by 0.5, reproducing the reference bit-exactly.

Per-core kernel (one batch sample per core):
  - one strided DMA gathers the 32x32 needed rows of 64 floats (256 KB) of
    channel struct_id into SBUF [128, 512]
  - DVE: add of the two W-phases (x[...,3::8] + x[...,4::8]) -> [128, 8, 8]
  - DVE: max-reduce over free dim -> [128, 1]
  - DMA out 128 partition maxima; host finishes max/mean over 8*128 values.

betti_error is 1 only for struct_id == 2 ('Myo'); for the other structures the
loss is exactly 0 and no device work is needed.
"""

import os

import numpy as np

_TARGETS = ((1, 0, 0), (1, 0, 0), (1, 1, 0), (1, 0, 0))
_BETTI_FALLBACK = (1, 0, 0)

_N_CORES = 8
_IN_SHAPE = (4, 160, 160, 64)  # per-sample [C, D, H, W]

_module_cache: dict = {}
LAST_RESULTS = None  # BassKernelResults of the most recent device run


def _patch_tail_drain():
    """walrus in this image rejects >1 sem wait on the SP tail Drain
    ("Too many sync wait commands", CoreV3GenImpl setupSyncWait).  Split
    Tile's single multi-wait tail drain into one drain per semaphore."""
    import concourse.tile as tile
    from concourse import mybir
    from concourse.vector_clock import ScopedClock

    if getattr(tile.TileContext, "_betti_drain_patch", False):
        return

    def _drain_and_barrier(self, tick_clock, wait_clock):
        drain_inst = self.nc.sync.drain()
        wait_clock.add_sem_waits(
            drain_inst.ins, ScopedClock({None: tick_clock.global_clock})
        )
        si = drain_inst.ins.sync_info
        if si is not None and si.on_wait and len(si.on_wait) > 1:
            waits = list(si.on_wait)
            drain_inst.ins.sync_info = mybir.SyncInfo(
                on_wait=[waits[0]], on_update=list(si.on_update or [])
            )
            for w in waits[1:]:
                extra = self.nc.sync.drain()
                extra.ins.sync_info = mybir.SyncInfo(on_wait=[w], on_update=[])
        self.nc.all_engine_barrier()
        assert self.sems is not None
        popped = self.nc._tile_sem_poison_stack.pop()
        assert popped is self._sem_poison
        self.nc.clear_and_free_semaphores(list(self.sems.allocated().values()))
        self.nc.all_engine_barrier()

    tile.TileContext._drain_and_barrier = _drain_and_barrier
    tile.TileContext._betti_drain_patch = True


def _build(struct_id: int):
    import concourse.bass as bass
    import concourse.tile as tile
    from concourse import mybir

    _patch_tail_drain()

    nc = bass.Bass("TRN2", target_bir_lowering=False, debug=False,
                   num_devices=_N_CORES)
    x = nc.dram_tensor("x", list(_IN_SHAPE), mybir.dt.float32,
                       kind="ExternalInput").ap()
    o = nc.dram_tensor("o", [1], mybir.dt.float32,
                       kind="ExternalOutput").ap()
    with tile.TileContext(nc) as tc:
        with tc.tile_pool(name="p", bufs=1) as pool:
            # 32 D-rows on partitions; j (H) split across the two HWDGE
            # engines so descriptor generation runs on two queues.
            sub = x[struct_id, 2::5, 2::5, :]          # [32, 32, 64] strided
            t_a = pool.tile([32, 1024], mybir.dt.float32)
            t_b = pool.tile([32, 1024], mybir.dt.float32)
            nc.sync.dma_start(t_a[:], sub[:, 0:16, :])
            nc.scalar.dma_start(t_b[:], sub[:, 16:32, :])
            va = t_a[:].rearrange("p (j w) -> p j w", w=64)
            vb = t_b[:].rearrange("p (j w) -> p j w", w=64)
            scr = pool.tile([32, 256], mybir.dt.float32)
            sa = scr[:, 0:128].rearrange("p (j k) -> p j k", k=8)
            sb = scr[:, 128:256].rearrange("p (j k) -> p j k", k=8)
            nc.vector.tensor_tensor(out=sa, in0=va[:, :, 3::8],
                                    in1=va[:, :, 4::8],
                                    op=mybir.AluOpType.add)
            nc.vector.tensor_tensor(out=sb, in0=vb[:, :, 3::8],
                                    in1=vb[:, :, 4::8],
                                    op=mybir.AluOpType.add)
            # Per-partition maxima into column 0, then a 32x32 block
            # transpose lands all 32 values in partition 0 so the output
            # DMA is a single 4-byte packet (a [32,1] partition-gather DMA
            # costs ~50 ns/packet in completion pacing).
            redp = pool.tile([32, 32], mybir.dt.float32)
            nc.vector.reduce_max(redp[:, 0:1], scr[:],
                                 axis=mybir.AxisListType.X)
            redt = pool.tile([32, 32], mybir.dt.float32)
            nc.vector.transpose(redt[:], redp[:])
            fin = pool.tile([32, 1], mybir.dt.float32)
            nc.vector.reduce_max(fin[0:1, :], redt[0:1, :],
                                 axis=mybir.AxisListType.X)
            nc.sync.dma_start(o[:], fin[0:1, :])
    return nc


def kernel(p_hat: np.ndarray, struct_id) -> np.ndarray:
    global LAST_RESULTS
    sid = int(struct_id)
    target = _TARGETS[sid]
    betti_error = sum(abs(_BETTI_FALLBACK[k] - target[k]) for k in range(3))
    B = p_hat.shape[0]
    if betti_error == 0:
        return np.zeros((), dtype=p_hat.dtype)

    from concourse import bass_utils

    assert B == _N_CORES and tuple(p_hat.shape[1:]) == _IN_SHAPE, (
        f"kernel hardcoded for shape (8, 4, 160, 160, 64), got {p_hat.shape}"
    )
    if sid not in _module_cache:
        _module_cache[sid] = _build(sid)
    nc = _module_cache[sid]

    p_hat = np.ascontiguousarray(p_hat, dtype=np.float32)
    in_maps = [{"x": p_hat[b]} for b in range(B)]
    trace = bool(int(os.environ.get("BETTI_TRACE", "0")))
    res = bass_utils.run_bass_kernel_spmd(
        nc, in_maps, core_ids=list(range(_N_CORES)), trace=trace
    )
    LAST_RESULTS = res

    per_core = np.stack([r["o"] for r in res.results])        # [8, 1]
    m = per_core.max(axis=1).astype(np.float32)               # max of (a+b)
    conf = np.float32(0.5) * m                                # exact scaling
    total = np.sum((np.float32(1.0) - conf) * np.float32(betti_error),
                   dtype=np.float32)
    out = total / np.float32(max(B, 1))
    return np.asarray(out, dtype=p_hat.dtype)


# revision 5
# speedup vs baseline: 1.7149x; 1.2327x over previous
"""Betti3D loss kernel for Trainium2 (8 NeuronCores, data-parallel over batch).

Reference computation (see problem):
    p_down  = trilinear_resize(p_hat, (32, 32, 8))   # [B, C, 32, 32, 8]
    conf[b] = max(p_down[b, struct_id])
    out     = sum((1 - conf) * betti_error) / B

With input [B, C, 160, 160, 64] -> (32, 32, 8) the resize scales are exactly
(5, 5, 8), so with torch/jax half-pixel centers the source coordinates are:
    D axis: 5*i + 2      (weight exactly 0 -> pure gather)
    H axis: 5*j + 2      (weight exactly 0 -> pure gather)
    W axis: 8*k + 3.5    (weight exactly 0.5 -> 0.5*(x[8k+3] + x[8k+4]))
Therefore
    p_down[b, c, i, j, k] = 0.5 * (x[b,c,5i+2,5j+2,8k+3] + x[b,c,5i+2,5j+2,8k+4])
and conf[b] = 0.5 * max_{i,j,k} (x[...,8k+3] + x[...,8k+4]).  Since scaling by
0.5 commutes with max (and is exact in fp32), the device kernel computes
max(a+b) and the host multiplies by 0.5, reproducing the reference bit-exactly.

Per-core kernel (one batch sample per core):
  - one strided DMA gathers the 32x32 needed rows of 64 floats (256 KB) of
    channel struct_id into SBUF [128, 512]
  - DVE: add of the two W-phases (x[...,3::8] + x[...,4::8]) -> [128, 8, 8]
  - DVE: max-reduce over free dim -> [128, 1]
  - DMA out 128 partition maxima; host finishes max/mean over 8*128 values.

betti_error is 1 only for struct_id == 2 ('Myo'); for the other structures the
loss is exactly 0 and no device work is needed.
"""

import os

import numpy as np

_TARGETS = ((1, 0, 0), (1, 0, 0), (1, 1, 0), (1, 0, 0))
_BETTI_FALLBACK = (1, 0, 0)

_N_CORES = 8
_IN_SHAPE = (4, 160, 160, 64)  # per-sample [C, D, H, W]

_module_cache: dict = {}
LAST_RESULTS = None  # BassKernelResults of the most recent device run


def _patch_tail_drain():
    """walrus in this image rejects >1 sem wait on the SP tail Drain
    ("Too many sync wait commands", CoreV3GenImpl setupSyncWait).  Split
    Tile's single multi-wait tail drain into one drain per semaphore."""
    import concourse.tile as tile
    from concourse import mybir
    from concourse.vector_clock import ScopedClock

    if getattr(tile.TileContext, "_betti_drain_patch", False):
        return

    def _drain_and_barrier(self, tick_clock, wait_clock):
        drain_inst = self.nc.sync.drain()
        wait_clock.add_sem_waits(
            drain_inst.ins, ScopedClock({None: tick_clock.global_clock})
        )
        si = drain_inst.ins.sync_info
        if si is not None and si.on_wait and len(si.on_wait) > 1:
            waits = list(si.on_wait)
            drain_inst.ins.sync_info = mybir.SyncInfo(
                on_wait=[waits[0]], on_update=list(si.on_update or [])
            )
            for w in waits[1:]:
                extra = self.nc.sync.drain()
                extra.ins.sync_info = mybir.SyncInfo(on_wait=[w], on_update=[])
        self.nc.all_engine_barrier()
        assert self.sems is not None
        popped = self.nc._tile_sem_poison_stack.pop()
        assert popped is self._sem_poison
        self.nc.clear_and_free_semaphores(list(self.sems.allocated().values()))
        self.nc.all_engine_barrier()

    tile.TileContext._drain_and_barrier = _drain_and_barrier
    tile.TileContext._betti_drain_patch = True


def _strip_const_memsets(m):
    """Bass.__init__ memsets four const-* SBUF tensors this kernel never
    reads; they are the first 'useful' instructions in the NTFF window and
    cost ~0.5 us on the Pool engine.  Drop them."""
    for function in m.functions:
        for block in function.blocks:
            keep = [
                inst
                for inst in block.instructions
                if not (
                    type(inst).__name__ == "InstMemset"
                    and inst.outs
                    and getattr(inst.outs[0], "memref", "").startswith("const-")
                )
            ]
            if len(keep) != len(block.instructions):
                block.instructions[:] = keep


def _build(struct_id: int):
    import concourse.bass as bass
    import concourse.tile as tile
    from concourse import mybir

    _patch_tail_drain()

    nc = bass.Bass("TRN2", target_bir_lowering=False, debug=False,
                   num_devices=_N_CORES)
    x = nc.dram_tensor("x", list(_IN_SHAPE), mybir.dt.float32,
                       kind="ExternalInput").ap()
    o = nc.dram_tensor("o", [4], mybir.dt.float32,
                       kind="ExternalOutput").ap()
    with tile.TileContext(nc) as tc:
        with tc.tile_pool(name="p", bufs=1) as pool:
            # 32 D-rows on partitions.  The 1024 strided 256B source rows
            # are descriptor-generation-bound, so split into 4 chunks over
            # the two HWDGE engines (SP + ACT) and pipeline the DVE work
            # per chunk under the remaining DMA time.
            sub = x[struct_id, 2::5, 2::5, :]          # [32, 32, 64] strided
            chunks = []
            for ci, (eng, j0) in enumerate(
                [(nc.sync, 0), (nc.scalar, 16), (nc.sync, 8), (nc.scalar, 24)]
            ):
                t_c = pool.tile([32, 512], mybir.dt.float32, tag=f"t{ci}")
                eng.dma_start(t_c[:], sub[:, j0:j0 + 8, :])
                chunks.append(t_c)
            scr = pool.tile([32, 256], mybir.dt.float32)
            redp = pool.tile([32, 32], mybir.dt.float32)
            for ci, t_c in enumerate(chunks):
                v = t_c[:].rearrange("p (j w) -> p j w", w=64)
                s = scr[:, ci * 64:(ci + 1) * 64].rearrange(
                    "p (j k) -> p j k", k=8)
                nc.vector.tensor_tensor(out=s, in0=v[:, :, 3::8],
                                        in1=v[:, :, 4::8],
                                        op=mybir.AluOpType.add)
                nc.vector.reduce_max(redp[:, ci:ci + 1],
                                     scr[:, ci * 64:(ci + 1) * 64],
                                     axis=mybir.AxisListType.X)
            # 32x32 block transpose lands the per-partition maxima of
            # chunk ci in partition ci, so the output DMA is 4 packets.
            redt = pool.tile([32, 32], mybir.dt.float32)
            nc.vector.transpose(redt[:], redp[:])
            fin = pool.tile([32, 1], mybir.dt.float32)
            nc.vector.reduce_max(fin[0:4, :], redt[0:4, :],
                                 axis=mybir.AxisListType.X)
            nc.sync.dma_start(o[:], fin[0:4, :])
    _strip_const_memsets(nc.m)
    return nc


def kernel(p_hat: np.ndarray, struct_id) -> np.ndarray:
    global LAST_RESULTS
    sid = int(struct_id)
    target = _TARGETS[sid]
    betti_error = sum(abs(_BETTI_FALLBACK[k] - target[k]) for k in range(3))
    B = p_hat.shape[0]
    if betti_error == 0:
        return np.zeros((), dtype=p_hat.dtype)

    from concourse import bass_utils

    assert B == _N_CORES and tuple(p_hat.shape[1:]) == _IN_SHAPE, (
        f"kernel hardcoded for shape (8, 4, 160, 160, 64), got {p_hat.shape}"
    )
    if sid not in _module_cache:
        _module_cache[sid] = _build(sid)
    nc = _module_cache[sid]

    p_hat = np.ascontiguousarray(p_hat, dtype=np.float32)
    in_maps = [{"x": p_hat[b]} for b in range(B)]
    trace = bool(int(os.environ.get("BETTI_TRACE", "0")))
    res = bass_utils.run_bass_kernel_spmd(
        nc, in_maps, core_ids=list(range(_N_CORES)), trace=trace
    )
    LAST_RESULTS = res

    per_core = np.stack([r["o"] for r in res.results])        # [8, 4]
    m = per_core.max(axis=1).astype(np.float32)               # max of (a+b)
    conf = np.float32(0.5) * m                                # exact scaling
    total = np.sum((np.float32(1.0) - conf) * np.float32(betti_error),
                   dtype=np.float32)
    out = total / np.float32(max(B, 1))
    return np.asarray(out, dtype=p_hat.dtype)
